# revision 21
# baseline (speedup 1.0000x reference)
"""Multi-head causal attention (B=4,T=2048,C=1024,H=16,HS=64) on 8 TRN2 cores.

Sharding: core c -> batch b=c//2, head-group hg=c%2 (8 heads each).
Each core computes QKV projections for its heads, causal flash-attention,
and a partial output projection over its 512 combo channels, emitting
out^T partial [1024, 2048].  Host sums the two partials per batch (the
tensor-parallel all-reduce) and transposes.

Matmuls run in bf16 (fp32 PSUM accumulation); softmax skips
max-subtraction (scores ~ N(0,1), exp never overflows); the softmax
denominator comes free as a 65th row of the PV matmul via a ones-column
appended to V.

Single fused pipeline paced by the ACT engine's exp stream: attention
for query block qb processes heads in pairs (head 2hp on PE rows 0-63,
head 2hp+1 on rows 64-127, concurrent via row-tiled matmuls into
separate PSUM banks), and the QKV projection groups of block qb+1 plus
the output-projection groups of block qb-1 are emitted between head
pairs as PE filler, so the PE never idles (HAM stays warm) and the ACT
never starves.
"""

import os
import sys

if "/opt/trn_rl_repo" not in sys.path:
    sys.path.insert(0, "/opt/trn_rl_repo")

import ml_dtypes
import numpy as np

import concourse.mybir as mybir
import concourse.tile as tile
from concourse import bacc
from concourse.bass_utils import run_bass_kernel_spmd

P = 128
B, T, C, H = 4, 2048, 1024, 16
HS = C // H              # 64
HL = H // 2              # 8 local heads per core
HD = HL * HS             # 512 local combo channels
NT = T // 512            # 4 query blocks of 512
NCC = C // P             # 8 contraction chunks over C
NKC = T // P             # 16 key chunks of 128
F32 = mybir.dt.float32
BF16 = mybir.dt.bfloat16
EXP_SCALE = float(HS) ** -0.5  # 1/8, folded into the exp activation

MM_DT = BF16
MM_NP = ml_dtypes.bfloat16

_PROGRAM = None


def _build_program():
    nc = bacc.Bacc("TRN2", target_bir_lowering=False, debug=False, num_devices=8)

    xT = nc.dram_tensor("xT", [C, T], MM_DT, kind="ExternalInput")
    wq = nc.dram_tensor("wq", [C, HD], MM_DT, kind="ExternalInput")
    wk = nc.dram_tensor("wk", [C, HD], MM_DT, kind="ExternalInput")
    wv = nc.dram_tensor("wv", [C, HD], MM_DT, kind="ExternalInput")
    wpT = nc.dram_tensor("wpT", [HD, C], MM_DT, kind="ExternalInput")
    bias = nc.dram_tensor("bias", [C], F32, kind="ExternalInput")
    masks = nc.dram_tensor("masks", [P, P], MM_DT, kind="ExternalInput")
    outT = nc.dram_tensor("outT", [C, T], F32, kind="ExternalOutput")

    Exp = mybir.ActivationFunctionType.Exp

    with tile.TileContext(nc) as tc:
        with (
            tc.tile_pool(name="persist", bufs=1) as persist,
            tc.tile_pool(name="xtp", bufs=2) as xtp,
            tc.tile_pool(name="ptp", bufs=4) as ptp,
            tc.tile_pool(name="misc", bufs=4) as misc,
            tc.tile_pool(name="outp", bufs=3) as outp,
            tc.tile_pool(name="ps_gen", bufs=2, space="PSUM") as ps_gen,
            tc.tile_pool(name="ps_s", bufs=1, space="PSUM") as ps_s,
            tc.tile_pool(name="ps_o", bufs=2, space="PSUM") as ps_o,
        ):
            # Q^T / K^T with head pairs stacked on partitions: chunk hp holds
            # head 2hp in rows 0-63 and head 2hp+1 in rows 64-127
            qt = persist.tile([P, HL // 2, T], MM_DT)
            kt = persist.tile([P, HL // 2, T], MM_DT)
            vaug = persist.tile([P, NKC, HL, HS + 1], MM_DT)
            bias_sb = persist.tile([P, C // P], F32)
            wq_sb = persist.tile([P, NCC, HD], MM_DT, tag="wq")
            wk_sb = persist.tile([P, NCC, HD], MM_DT, tag="wk")
            wv_sb = persist.tile([P, NCC, HD], MM_DT, tag="wv")
            wpT_sb = persist.tile([P, HD // P, C], MM_DT, tag="wpT")
            masks_sb = persist.tile([P, P], MM_DT, tag="masks")
            comboT = persist.tile([P, HD // P, T], MM_DT, tag="comboT")

            # one score psum per pair half, no per-head ping-pong: the
            # head pair itself provides the PE/ACT overlap (PE fills the
            # exp window of one half with the other half + filler MMs)
            pssA = ps_s.tile([P, 2, 512], F32, tag="pssA", name="pssA")
            pssB = ps_s.tile([P, 2, 512], F32, tag="pssB", name="pssB")

            # ones column for the softmax-denominator row of the PV matmul
            nc.vector.memset(vaug[:, :, :, HS : HS + 1], 1.0)
            # setup loads: wk/wq arrive as per-head-pair column slices so the
            # first QK projection groups can start as early as possible;
            # wv (needed in full by the first V group) gets its own queue;
            # wpT/bias/masks are needed late and go last
            xts = {}

            def dma_x(tb, queues=(nc.sync, nc.sync)):
                tsl = slice(tb * 512, (tb + 1) * 512)
                xt = xtp.tile([P, NCC, 512], MM_DT, tag="xt")
                for _h in range(2):
                    queues[_h].dma_start(
                        xt[:, 4 * _h : 4 * _h + 4, :],
                        xT[:].rearrange("(co p) t -> p co t", p=P)[:, 4 * _h : 4 * _h + 4, tsl],
                    )
                xts[tb] = xt

            def qkv_qk_group(tb, hb):
                """Project Q and K for head-pair block hb of time block tb."""
                tsl = slice(tb * 512, (tb + 1) * 512)
                hsl = slice(hb * P, (hb + 1) * P)
                xt = xts[tb]
                for w_sb, dst in ((wk_sb, kt), (wq_sb, qt)):
                    pqk = ps_gen.tile([P, 512], F32, tag="pqk")
                    for co in range(NCC):
                        nc.tensor.matmul(
                            pqk[:],
                            w_sb[:, co, hsl],
                            xt[:, co, :],
                            start=(co == 0),
                            stop=(co == NCC - 1),
                        )
                    nc.vector.tensor_copy(out=dst[:, hb, tsl], in_=pqk[:])

            def qkv_v_group(tb, ts2):
                """Project V for token sub-chunk ts2 of time block tb."""
                xt = xts[tb]
                pv = ps_gen.tile([P, 512], F32, tag="pqk")
                for co in range(NCC):
                    nc.tensor.matmul(
                        pv[:],
                        xt[:, co, ts2 * P : (ts2 + 1) * P],
                        wv_sb[:, co, :],
                        start=(co == 0),
                        stop=(co == NCC - 1),
                    )
                kc = tb * 4 + ts2
                nc.vector.tensor_copy(
                    out=vaug[:, kc, :, 0:HS],
                    in_=pv[:].rearrange("p (h d) -> p h d", h=HL),
                )

            def proj_group(qb, db):
                """One 128-row chunk of the output projection for block qb."""
                q0 = qb * 512
                pp = ps_gen.tile([P, 512], F32, tag="pqk")
                for co in range(HD // P):
                    nc.tensor.matmul(
                        pp[:],
                        wpT_sb[:, co, db * P : (db + 1) * P],
                        comboT[:, co, q0 : q0 + 512],
                        start=(co == 0),
                        stop=(co == HD // P - 1),
                    )
                ot = outp.tile([P, 512], F32, tag="ot")
                nc.vector.tensor_scalar_add(ot[:], pp[:], bias_sb[:, db : db + 1])
                nc.sync.dma_start(outT[db * P : (db + 1) * P, q0 : q0 + 512], ot[:])

            # proj(NT-1) is split so its first contraction half runs while
            # the last block's exp stream is still going; only the second
            # half + add + store remain after the final attention pair
            ot1 = persist.tile([P, C // P, 512], F32, tag="ot1")

            def proj_last_half1(db):
                q0 = (NT - 1) * 512
                pp = ps_gen.tile([P, 512], F32, tag="pqk")
                for co in range(2):
                    nc.tensor.matmul(
                        pp[:],
                        wpT_sb[:, co, db * P : (db + 1) * P],
                        comboT[:, co, q0 : q0 + 512],
                        start=(co == 0),
                        stop=(co == 1),
                    )
                nc.vector.tensor_scalar_add(
                    ot1[:, db, :], pp[:], bias_sb[:, db : db + 1]
                )

            def proj_last_half2(db):
                q0 = (NT - 1) * 512
                pp = ps_gen.tile([P, 512], F32, tag="pqk")
                for co in range(2, 4):
                    nc.tensor.matmul(
                        pp[:],
                        wpT_sb[:, co, db * P : (db + 1) * P],
                        comboT[:, co, q0 : q0 + 512],
                        start=(co == 2),
                        stop=(co == 3),
                    )
                ot = outp.tile([P, 512], F32, tag="ot")
                nc.vector.tensor_add(out=ot[:], in0=pp[:], in1=ot1[:, db, :])
                nc.sync.dma_start(outT[db * P : (db + 1) * P, q0 : q0 + 512], ot[:])

            def attention_pair(qb, hp, fillers=()):
                """Causal attention for heads (2hp, 2hp+1) on query block qb.

                Head 2hp runs on PE rows 0-63 into pssA, head 2hp+1 on rows
                64-127 into pssB; the row-tiled score matmuls execute
                concurrently.  Diagonal-chunk matmuls skip fully-masked
                column prefixes (psum there holds bounded stale scores —
                exp'd then zeroed by the mask / skipped by the PV c0),
                except the very first generation which must initialize the
                banks full-width.
                """
                q0 = qb * 512
                last_kc = qb * 4 + 3
                poAB = [
                    ps_o.tile([P, 512], F32, tag="po", name=f"po{_j}")
                    for _j in range(2)
                ]
                for g in range(2 * (qb + 1)):
                    for i in range(2):
                        kc = 2 * g + i
                        mi = kc - 4 * qb
                        c0 = kc * P - q0 if (mi >= 0 and (qb > 0 or hp > 0)) else 0
                        for half, pss in ((0, pssA), (1, pssB)):
                            r0 = 64 * half
                            nc.tensor.matmul(
                                pss[:, i, c0:512],
                                kt[r0 : r0 + 64, hp, kc * P : (kc + 1) * P],
                                qt[r0 : r0 + 64, hp, q0 + c0 : q0 + 512],
                                start=True,
                                stop=True,
                                tile_position=(r0, 0),
                            )
                    pts = []
                    for half, pss in ((0, pssA), (1, pssB)):
                        pt = ptp.tile([P, 2, 512], MM_DT, tag="pt")
                        nc.scalar.activation(pt[:], pss[:], Exp, scale=EXP_SCALE)
                        for i in range(2):
                            kc = 2 * g + i
                            mi = kc - 4 * qb
                            if mi >= 0:
                                # only the 128x128 diagonal block needs the
                                # triangle mask: columns left of it are
                                # suffix-skipped in the PV matmul, columns
                                # right of it are fully unmasked
                                msl = slice(mi * P, (mi + 1) * P)
                                nc.vector.tensor_mul(
                                    out=pt[:, i, msl],
                                    in0=pt[:, i, msl],
                                    in1=masks_sb[:],
                                )
                        pts.append(pt)
                    # filler work (QKV projections of a later block, output
                    # projections of an earlier one) lands between this
                    # group's exp and its PV, exactly where the PE would
                    # otherwise stall waiting on the ACT engine
                    if g < len(fillers):
                        for emit in fillers[g]:
                            emit()
                    for half in range(2):
                        h = 2 * hp + half
                        pt, po = pts[half], poAB[half]
                        for i in range(2):
                            kc = 2 * g + i
                            mi = kc - 4 * qb
                            c0 = max(0, kc * P - q0) if mi >= 0 else 0
                            nc.tensor.matmul(
                                po[0 : HS + 1, c0:512],
                                vaug[:, kc, h, :],
                                pt[:, i, c0:512],
                                start=(kc == 0),
                                stop=(kc == last_kc),
                            )
                # normalize rows by the denominator row (65th) of po;
                # custom-DVE reciprocal requires partition-0 input, so
                # stage the row via a copy first
                for half in range(2):
                    h = 2 * hp + half
                    po = poAB[half]
                    den = misc.tile([1, 512], F32, tag="den")
                    nc.vector.tensor_copy(out=den[:], in_=po[HS : HS + 1, :])
                    rc = misc.tile([1, 512], F32, tag="rc")
                    nc.vector.reciprocal_approx_fast(rc[:], den[:])
                    rb = misc.tile([HS, 512], F32, tag="rb")
                    nc.gpsimd.partition_broadcast(rb[:], rc[:])
                    nc.vector.tensor_mul(
                        out=comboT[(h % 2) * 64 : (h % 2) * 64 + 64, h // 2, q0 : q0 + 512],
                        in0=po[0:HS, :],
                        in1=rb[:],
                    )

            # ---- fused pipeline ----
            # Filler placement: QKV groups of block tb+1 are spread over the
            # attention pairs of block tb (finishing before tb+1 starts);
            # output projections are deferred as late as dependencies allow
            # because qb=3 is ACT-bound (its exp stream outweighs its own
            # matmuls) and the deferred groups keep the PE fed there.
            def qk(tb, hb):
                return lambda: qkv_qk_group(tb, hb)

            def vg(tb, ts2):
                return lambda: qkv_v_group(tb, ts2)

            def pj(qb, db):
                return lambda: proj_group(qb, db)

            # setup loads: the first QK group needs the hb=0 column slice of
            # wk/wq plus the whole first x block, so those go first, spread
            # across the three DMA-capable queues
            wkr = wk[:].rearrange("(co p) n -> p co n", p=P)
            wqr = wq[:].rearrange("(co p) n -> p co n", p=P)
            nc.gpsimd.dma_start(wk_sb[:, :, 0:P], wkr[:, :, 0:P])
            nc.scalar.dma_start(wq_sb[:, :, 0:P], wqr[:, :, 0:P])
            dma_x(0, queues=(nc.sync, nc.scalar))
            nc.gpsimd.dma_start(wk_sb[:, :, P:HD], wkr[:, :, P:HD])
            nc.scalar.dma_start(wq_sb[:, :, P:HD], wqr[:, :, P:HD])
            wvr = wv[:].rearrange("(co p) n -> p co n", p=P)
            nc.gpsimd.dma_start(wv_sb[:, 0:4, :], wvr[:, 0:4, :])
            nc.sync.dma_start(wv_sb[:, 4:8, :], wvr[:, 4:8, :])
            nc.gpsimd.dma_start(masks_sb[:], masks[:])
            nc.sync.dma_start(wpT_sb[:], wpT[:].rearrange("(co p) n -> p co n", p=P))
            nc.sync.dma_start(bias_sb[:], bias[:].rearrange("(db p) -> p db", p=P))

            qkv_qk_group(0, 0)
            dma_x(1)
            # pair (0,0) is special: its PV genuinely depends on the V
            # groups of block 0, so they are emitted inside the pair
            # (between exp and PV); everywhere else filler work is emitted
            # AFTER the pair so the scheduler gives the attention chain
            # priority and uses the filler only to plug PE idle.
            post = {
                (0, 0): [qk(0, 1)],
                (0, 1): [qk(0, 2), qk(1, 0)],
                (0, 2): [qk(0, 3), vg(1, 0), vg(1, 1), qk(1, 1)],
                (0, 3): [vg(1, 2), vg(1, 3), qk(1, 2)],
                (1, 0): [qk(1, 3), qk(2, 0), vg(2, 0)],
                (1, 1): [qk(2, 1), vg(2, 1)],
                (1, 2): [qk(2, 2), vg(2, 2)],
                (1, 3): [qk(2, 3), vg(2, 3)],
                (2, 0): [qk(3, 0), vg(3, 0), pj(0, 0), pj(0, 1)],
                (2, 1): [qk(3, 1), vg(3, 1), pj(0, 2), pj(0, 3)],
                (2, 2): [qk(3, 2), vg(3, 2), pj(0, 4), pj(0, 5)],
                (2, 3): [qk(3, 3), vg(3, 3), pj(0, 6), pj(0, 7)],
            }

            # qb=3 is ACT-bound throughout, so the deferred projection work
            # is front-loaded: everything that is dependency-ready goes into
            # the earliest pair, leaving only the half2 combine after the
            # final pair.  (half1 groups need comboT chunks 0-1 = pairs 0-1.)
            post[(3, 0)] = [pj(1, db) for db in range(8)]
            post[(3, 1)] = [pj(2, db) for db in range(8)]
            post[(3, 2)] = [lambda db=db: proj_last_half1(db) for db in range(8)]
            post[(3, 3)] = []
            first_fill = [[vg(0, 0), vg(0, 1)], [vg(0, 2), vg(0, 3)]]
            for qb in range(NT):
                for hp in range(4):
                    fillers = first_fill if (qb, hp) == (0, 0) else ()
                    attention_pair(qb, hp, fillers=fillers)
                    for emit in post[(qb, hp)]:
                        emit()
                if qb + 2 <= NT - 1:
                    dma_x(qb + 2)
            for db in range(C // P):
                proj_last_half2(db)

    nc.finalize()
    return nc


def _causal_masks():
    # lower-triangle [128,128]: 1.0 iff kl <= ql (applied multiplicatively
    # post-exp to the single diagonal block of each diagonal key chunk)
    kl = np.arange(P)[:, None]
    ql = np.arange(P)[None, :]
    return (kl <= ql).astype(np.float32)


def _in_maps(x, Wq, Wk, Wv, Wproj, bproj):
    masks = _causal_masks()
    zeros_bias = np.zeros_like(bproj)
    maps = []
    for core in range(8):
        b, hg = core // 2, core % 2
        hs = slice(hg * HL, (hg + 1) * HL)
        maps.append(
            {
                "xT": np.ascontiguousarray(x[b].T).astype(MM_NP),
                "wq": np.ascontiguousarray(
                    Wq[hs].transpose(1, 0, 2).reshape(C, HD).astype(MM_NP)
                ),
                "wk": np.ascontiguousarray(
                    Wk[hs].transpose(1, 0, 2).reshape(C, HD).astype(MM_NP)
                ),
                "wv": np.ascontiguousarray(
                    Wv[hs].transpose(1, 0, 2).reshape(C, HD).astype(MM_NP)
                ),
                "wpT": np.ascontiguousarray(Wproj[:, hg * HD : (hg + 1) * HD].T).astype(MM_NP),
                "bias": np.ascontiguousarray(bproj if hg == 0 else zeros_bias),
                "masks": masks.astype(MM_NP),
            }
        )
    return maps


def get_program():
    global _PROGRAM
    if _PROGRAM is None:
        _PROGRAM = _build_program()
    return _PROGRAM


def kernel(x, Wq, Wk, Wv, Wproj, bproj, _run_kwargs=None):
    x = np.asarray(x, dtype=np.float32)
    Wq = np.asarray(Wq, dtype=np.float32)
    Wk = np.asarray(Wk, dtype=np.float32)
    Wv = np.asarray(Wv, dtype=np.float32)
    Wproj = np.asarray(Wproj, dtype=np.float32)
    bproj = np.asarray(bproj, dtype=np.float32)

    nc = get_program()
    res = run_bass_kernel_spmd(
        nc,
        _in_maps(x, Wq, Wk, Wv, Wproj, bproj),
        core_ids=list(range(8)),
        **(_run_kwargs or {}),
    )
    out = np.empty((B, T, C), dtype=np.float32)
    for b in range(B):
        out[b] = (res.results[2 * b]["outT"] + res.results[2 * b + 1]["outT"]).T
    kernel.last_results = res
    return out
